# revision 37
# baseline (speedup 1.0000x reference)
"""Single-head causal attention with softmax over the QUERY axis (dim=1).

out[b,i,d] = sum_j softmax_i(mask(q@kT/8))[i,j] * v[j,d]

Data-parallel over batch B=8, one batch element per NeuronCore.

Per-core pipeline (all projection/score/av matmuls bf16 at 1 cycle/row):
  - x arrives in 4 s-groups of 512 rows issued back-to-back, group 3
    first; s-tile 15 has its own small DMA so phase A starts ~3.5us in.
    The bf16 identity and causal mask are generated on-chip (GPSIMD
    affine_select) to keep head DMAs off the critical path.
  - phase A runs PER S-TILE, software-pipelined ahead of the attention
    steps (transposes 3 tiles ahead, projections 2 ahead) so each
    PE->DVE->PE round-trip hides behind independent PE work:
    PE-transpose x-tile -> xT (bf16), project qT|kT into one merged
    PSUM bank (the k chain uses start=False overwrite-by-has_written
    so a single DVE copy evacuates both), project v.
  - key tiles jt processed DESCENDING 15..0 (tile jt only needs phase A
    of tiles >= jt, which ran already).
  - scoresT[j,i] = kT_jt.T @ qT in 1024-col PSUM chunks; causal mask via
    a PE matmul (identity @ negmask accumulated into the diagonal bank);
    one Exp per chunk with accum_out producing the softmax denominator;
    vs[j,:] = v[j,:]/den[j] on GPSIMD (DVE for the last tiles); out[i,:]
    += attnT.T @ vs accumulated across jt in two independent single-bank
    PSUM accumulators (halves never serialize against each other).
  - output is bf16 (converted to f32 on host) and drains in 2 halves:
    bank A on DVE right after jt0's it<=7 matmuls, bank B on the (idle)
    scalar engine, overlapping the final DMA setup.
"""

import numpy as np
import sys

sys.path.insert(0, "/opt/trn_rl_repo")

import ml_dtypes
import concourse.mybir as mybir
from concourse.bacc import Bacc
from concourse.tile import TileContext
from concourse.bass_utils import run_bass_kernel_spmd

B, S, C, D = 8, 2048, 384, 64
P = 128
NT = S // P   # 16 s-tiles
CC = C // P   # 3 contraction chunks
F32 = mybir.dt.float32
F32R = mybir.dt.float32r
BF16 = mybir.dt.bfloat16
AFT = mybir.ActivationFunctionType
AX = mybir.AxisListType

CHW = 1024    # exp/psum chunk width (2 banks)

_COMPILED = None


def build_nc():
    nc = Bacc()
    x_b = nc.declare_dram_parameter("x_b", [S, C], F32R, isOutput=False)
    wqk = nc.declare_dram_parameter("wqk", [C, P], BF16, isOutput=False)  # [Wq/8 | Wk]
    wv = nc.declare_dram_parameter("wv", [C, D], BF16, isOutput=False)
    ident = nc.declare_dram_parameter("ident", [P, P], F32R, isOutput=False)
    out_b = nc.declare_dram_parameter("out_b", [S, D], BF16, isOutput=True)

    with TileContext(nc) as tc:
        with (
            tc.tile_pool(name="consts", bufs=1) as consts,
            tc.tile_pool(name="big", bufs=1) as big,
            tc.tile_pool(name="xsp", bufs=1) as xsp,
            tc.tile_pool(name="attnp", bufs=4) as attnp,
            tc.tile_pool(name="small", bufs=8) as small,
            tc.tile_pool(name="vsp", bufs=4) as vsp,
            tc.tile_pool(name="psO", bufs=1, space="PSUM") as psO,
            tc.tile_pool(name="psS", bufs=2, space="PSUM") as psS,
            tc.tile_pool(name="psA", bufs=2, space="PSUM") as psA,
        ):
            # ---- constants ----
            idt = consts.tile([P, P], F32R)
            idb = consts.tile([P, P], BF16)
            msk = consts.tile([P, P], BF16)
            ones = consts.tile([P, P], BF16)
            wqk_t = consts.tile([P, CC * P], BF16)
            wv_t = consts.tile([P, CC * D], BF16)
            trash = consts.tile([1, 2], F32)

            # preload the Exp table while DMAs are still in flight
            nc.vector.memset(trash, 0.0)
            nc.scalar.activation(trash[:, 0:1], trash[:, 1:2], AFT.Exp)

            # identities + causal mask generated on-chip (GPSIMD is idle and
            # this keeps the head DMAs off the critical path):
            # iota = q - p, so is_equal -> identity, is_ge -> causal keep.
            ISEL = dict(pattern=[[1, P]], base=0, channel_multiplier=-1)
            zer = consts.tile([P, P], BF16)
            nc.gpsimd.memset(ones, 1.0)
            nc.gpsimd.memset(zer, 0.0)
            nc.gpsimd.affine_select(
                idb, ones, compare_op=mybir.AluOpType.is_equal, fill=0.0, **ISEL)
            nc.gpsimd.affine_select(
                msk, zer, compare_op=mybir.AluOpType.is_ge, fill=-1e30, **ISEL)
            # x staging: s-tile 15 alone first, then 12..14, then groups
            xs_h = xsp.tile([P, C], F32R, tag="xsh", bufs=1)
            nc.sync.dma_start(out=xs_h, in_=x_b[15 * P:16 * P, :])
            # the fp32r identity must be DMA-sourced: the walrus verifier
            # rejects fp32r matmul operands from non-fp32r producers.
            nc.sync.dma_start(out=idt, in_=ident[:, :])
            nc.sync.dma_start(
                out=wqk_t.rearrange("p (c d) -> p c d", c=CC),
                in_=wqk.ap().rearrange("(c p) d -> p c d", p=P),
            )
            xs_r = xsp.tile([P, 3 * C], F32R, tag="xsr", bufs=1)
            nc.sync.dma_start(
                out=xs_r.rearrange("p (t c) -> p t c", t=3),
                in_=x_b[12 * P:15 * P, :].rearrange("(t p) c -> p t c", p=P),
            )
            nc.sync.dma_start(
                out=wv_t.rearrange("p (c d) -> p c d", c=CC),
                in_=wv.ap().rearrange("(c p) d -> p c d", p=P),
            )
            xs_g = {}
            for g in (2, 1, 0):
                xs_g[g] = xsp.tile([P, 4 * C], F32R, tag="xs", bufs=3,
                                   name=f"xs{g}")
                nc.sync.dma_start(
                    out=xs_g[g].rearrange("p (t c) -> p t c", t=4),
                    in_=x_b[g * 4 * P:(g + 1) * 4 * P, :].rearrange(
                        "(t p) c -> p t c", p=P),
                )

            # ---- persistent SBUF tensors ----
            xT = big.tile([P, CC * S], BF16)       # [128, 3*2048] xT (bf16)
            qk_sb = big.tile([64, 2 * S], BF16)    # qT (pre-scaled 1/8) | kT
            v_all = big.tile([P, NT * D], BF16)    # v tiles [128, 16*64]
            out_sb = big.tile([P, NT * D], BF16)   # final out staging

            # output accumulator split into two independent banks so the
            # bank-A drain never serializes against bank-B matmuls
            outpA = psO.tile([P, 8 * D], F32, tag="oA")
            outpB = psO.tile([P, 8 * D], F32, tag="oB")

            def xsrc(st, c):
                """x staging slice for s-tile st, contraction chunk c."""
                if st == 15:
                    return xs_h[:, c * P:(c + 1) * P]
                if st >= 12:
                    t = st - 12
                    return xs_r[:, t * C + c * P: t * C + (c + 1) * P]
                g, t = st // 4, st % 4
                return xs_g[g][:, t * C + c * P: t * C + (c + 1) * P]

            def emit_A1(st):
                """phase A stage 1 for one s-tile: transpose x -> xT."""
                pt = psA.tile([P, CC * P], F32, tag="pt", bufs=1,
                              name=f"pt{st}")
                for c in range(CC):
                    nc.tensor.matmul(
                        pt[:, c * P:(c + 1) * P].bitcast(F32R),
                        xsrc(st, c), idt,
                        is_transpose=True, start=(c == 0), stop=(c == CC - 1),
                    )
                dst = xT.rearrange("p (c s) -> p c s", c=CC)[
                    :, :, st * P:(st + 1) * P]
                srcv = pt.rearrange("p (c q) -> p c q", c=CC)
                if 11 <= st <= 14:
                    # ACT is exp-starved this early; use it for the copy so
                    # DVE latency stays off the qk-projection chain
                    nc.scalar.copy(dst, srcv)
                else:
                    nc.vector.tensor_copy(dst, srcv)

            def emit_A2(st):
                """phase A stage 2 for one s-tile: project v, qT and kT into
                ONE shared PSUM bank. The v chain's start=True clears the
                bank; the q and k chains land in their (unwritten) column
                ranges via the per-element has_written overwrite. One
                allocation per step keeps the ring reuse a full step apart."""
                pa = psA.tile([P, 512], F32, tag="aqv", bufs=1,
                              name=f"pa{st}")
                for c in range(CC):
                    nc.tensor.matmul(
                        pa[:, 0:D],
                        xT[:, c * S + st * P: c * S + (st + 1) * P],
                        wv_t[:, c * D:(c + 1) * D],
                        start=(c == 0), stop=False, skip_group_check=True,
                    )
                for c in range(CC):
                    nc.tensor.matmul(
                        pa[0:64, D:D + P], wqk_t[:, c * P: c * P + 64],
                        xT[:, c * S + st * P: c * S + (st + 1) * P],
                        start=False, stop=False, skip_group_check=True,
                    )
                for c in range(CC):
                    nc.tensor.matmul(
                        pa[0:64, D + P:D + 2 * P],
                        wqk_t[:, c * P + 64: c * P + 128],
                        xT[:, c * S + st * P: c * S + (st + 1) * P],
                        start=False, stop=(c == CC - 1), skip_group_check=True,
                    )
                nc.vector.tensor_copy(v_all[:, st * D:(st + 1) * D], pa[:, 0:D])
                nc.vector.tensor_copy(
                    qk_sb.rearrange("p (h s) -> p h s", h=2)
                         [:, :, st * P:(st + 1) * P],
                    pa[0:64, D:D + 2 * P].rearrange("p (h q) -> p h q", h=2),
                )

            def emit_scores(jt):
                """scoresT chunks + mask + Exp; returns (atile, dens, nch)."""
                Ni = S - jt * P
                atile = attnp.tile([P, S], BF16, tag="attn", name=f"atile{jt}")
                dens = small.tile([P, 2], F32, tag="dens", name=f"dens{jt}")
                nch = (Ni + CHW - 1) // CHW
                for ci in range(nch):
                    w = min(CHW, Ni - ci * CHW)
                    i0 = jt * P + ci * CHW
                    sc = psS.tile([P, CHW], F32, tag="psS", name=f"sc{jt}_{ci}")
                    for sub in range((w + 511) // 512):
                        sw = min(512, w - sub * 512)
                        diag = ci == 0 and sub == 0
                        nc.tensor.matmul(
                            sc[:, sub * 512: sub * 512 + sw],
                            qk_sb[:, S + jt * P: S + (jt + 1) * P],
                            qk_sb[:, i0 + sub * 512: i0 + sub * 512 + sw],
                            start=True, stop=not diag,
                        )
                    if ci == 0:
                        # causal mask on the diagonal block: += -1e30 (i < j)
                        nc.tensor.matmul(
                            sc[:, 0:P], idb, msk, start=False, stop=True)
                    nc.scalar.activation(
                        atile[:, ci * CHW: ci * CHW + w], sc[:, :w], AFT.Exp,
                        accum_out=dens[:, ci: ci + 1],
                    )
                return atile, dens, nch

            def emit_tail(jt, atile, dens, nch, last=False):
                if nch == 1:
                    den = dens[:, 0:1]
                else:
                    den_t = small.tile([P, 1], F32, tag="den", name=f"den{jt}")
                    nc.vector.reduce_sum(den_t, dens[:, :nch], axis=AX.X)
                    den = den_t
                rv = small.tile([P, 1], F32, tag="rv", name=f"rv{jt}")
                nc.vector.reciprocal(rv, den)
                vs = vsp.tile([P, D], BF16, tag="vs", name=f"vs{jt}")
                eng = nc.vector if jt <= 1 else nc.gpsimd
                eng.tensor_scalar_mul(vs, v_all[:, jt * D:(jt + 1) * D], rv)

                def av(it, stop=False):
                    if it < 8:
                        dst = outpA[:, it * D:(it + 1) * D]
                    else:
                        dst = outpB[:, (it - 8) * D:(it - 7) * D]
                    nc.tensor.matmul(
                        dst, atile[:, (it - jt) * P:(it - jt + 1) * P], vs,
                        start=(jt == 15 and it == 15) or (jt == 7 and it == 7),
                        stop=stop,
                    )

                if not last:
                    for it in range(jt, NT):
                        av(it)
                else:
                    # final jt: drain bank A (s 0..1023) as soon as it's done
                    for it in range(jt, 8):
                        av(it, stop=(it == 7))
                    nc.vector.tensor_copy(out_sb[:, 0:512], outpA)
                    nc.sync.dma_start(
                        out=out_b[0:8 * P, :].rearrange("(t p) d -> p t d", p=P),
                        in_=out_sb[:, 0:512].rearrange("p (t d) -> p t d", t=8),
                    )
                    for it in range(8, NT):
                        av(it, stop=(it == 15))
                    # bank B drains via the (now idle) scalar engine
                    nc.scalar.copy(out_sb[:, 512:1024], outpB)
                    nc.sync.dma_start(
                        out=out_b[8 * P:16 * P, :].rearrange(
                            "(t p) d -> p t d", p=P),
                        in_=out_sb[:, 512:1024].rearrange("p (t d) -> p t d", t=8),
                    )

            # ---- software-pipelined main loop ----
            # Transposes (A1) run 3 tiles ahead and projections (A2) 2 tiles
            # ahead of the scores, so each PE->DVE->PE round-trip (xT copy,
            # qk copy) has a full step of independent PE work behind it.
            seq = list(range(15, -1, -1))
            emit_A1(seq[0])
            emit_A1(seq[1])
            emit_A1(seq[2])
            emit_A2(seq[0])
            emit_A2(seq[1])
            pend = None
            for idx, jt in enumerate(seq):
                cur = (jt, *emit_scores(jt))
                if idx + 3 < NT:
                    emit_A1(seq[idx + 3])
                if idx + 2 < NT:
                    emit_A2(seq[idx + 2])
                if pend is not None:
                    pjt, atile, dens, nch = pend
                    emit_tail(pjt, atile, dens, nch)
                pend = cur
            pjt, atile, dens, nch = pend
            emit_tail(pjt, atile, dens, nch, last=True)

    nc.finalize()
    return nc


def _bf16(a):
    return np.ascontiguousarray(np.asarray(a, dtype=np.float32)).astype(
        ml_dtypes.bfloat16)


def _build_inputs(x, Wq, Wk, Wv):
    x = np.ascontiguousarray(np.asarray(x, dtype=np.float32))
    wq_s = np.asarray(Wq, dtype=np.float32) * np.float32(D ** -0.5)
    wqk = _bf16(np.concatenate([wq_s, np.asarray(Wk, dtype=np.float32)], axis=1))
    wv_ = _bf16(Wv)
    ident = np.eye(P, dtype=np.float32)
    return [
        {"x_b": x[b], "wqk": wqk, "wv": wv_, "ident": ident}
        for b in range(B)
    ]


def kernel(x, Wq, Wk, Wv, _trace=False):
    global _COMPILED
    if _COMPILED is None:
        _COMPILED = build_nc()
    nc = _COMPILED
    in_maps = _build_inputs(x, Wq, Wk, Wv)
    res = run_bass_kernel_spmd(nc, in_maps, core_ids=list(range(B)), trace=_trace)
    out = np.stack(
        [np.asarray(res.results[b]["out_b"]).astype(np.float32) for b in range(B)],
        axis=0)
    if _trace:
        kernel.last_results = res
    return out
